# revision 15
# baseline (speedup 1.0000x reference)
"""EMA recurrence kernel for Trainium2 (8 NeuronCores, Bass/Tile).

Computes a_t = w * x_t + (1 - w) * a_{t-1} over inputs [B=32, T=8192, C=128],
initial_state [B, C], weights [C] -> output [B, T, C].

Strategy (v3 — stride-2 decimated scan, multi-engine):
  - Pure data parallelism: batch dim sharded 4-per-core across 8 cores.
  - Host pre-transposes x to [B, C, T], casts to fp16, and de-interleaves
    even/odd time steps: channels map onto SBUF partitions, no on-chip
    transposes, and HBM traffic is halved vs fp32.
  - The recurrence is decimated by 2 to halve the (serial-throughput-
    limited) DVE scan work:
        a_{2j}   = c^2 * a_{2j-2} + u[j],   u[j] = c*w*x_{2j-1} + w*x_{2j}
        a_{2j+1} = c * a_{2j} + w * x_{2j+1}
  - Engine placement (per [C, J] chunk):
      PE    u' = diag(cw)@x_odd_shifted + diag(w)@x_even  -> PSUM (f32)
      DVE   tensor_tensor_scan(c^2, u') straight out of PSUM -> y_even fp16
      DVE/ACT  cae = c * y_even (per-partition scale, split for balance)
      ACT   wxo = w * x_odd
      GPSIMD y_odd = cae + wxo (tensor_tensor)
      DMA   in on SP ring, out on ACT ring
  - The scan runs in the output domain (a, not a/w), so fp16 magnitudes
    are bounded by the output and w==0 / w==1 channels are exact by
    construction (chunk init a_0/c with c_safe; c^2*(a0/c) == c*a0).
"""

import sys

if "/opt/trn_rl_repo" not in sys.path:
    sys.path.insert(0, "/opt/trn_rl_repo")

import numpy as np

B, T, C = 32, 8192, 128
NCORES = 8
BL = B // NCORES      # batches per core
T2 = T // 2           # even/odd stream length
J = 2048              # scan columns per chunk
NCH = T2 // J         # chunks per batch (2)
MM = 512              # matmul slice (one PSUM bank of f32)

_NC_CACHE = None


def build_bass():
    global _NC_CACHE
    if _NC_CACHE is not None:
        return _NC_CACHE

    import concourse.bacc as bacc
    import concourse.mybir as mybir
    import concourse.tile as tile

    f32 = mybir.dt.float32
    f16 = mybir.dt.float16
    AF = mybir.ActivationFunctionType
    ALU = mybir.AluOpType

    nc = bacc.Bacc("TRN2", target_bir_lowering=False, debug=False)
    xe = nc.dram_tensor("xe", [BL, C, T2], f16, kind="ExternalInput").ap()
    xo = nc.dram_tensor("xo", [BL, C, T2], f16, kind="ExternalInput").ap()
    s0q = nc.dram_tensor("s0q", [C, BL], f32, kind="ExternalInput").ap()
    c2dec = nc.dram_tensor("c2dec", [C, J], f16, kind="ExternalInput").ap()
    ccol = nc.dram_tensor("ccol", [C, 1], f32, kind="ExternalInput").ap()
    wcol = nc.dram_tensor("wcol", [C, 1], f32, kind="ExternalInput").ap()
    cwdiag = nc.dram_tensor("cwdiag", [128, 128], f16, kind="ExternalInput").ap()
    wdiag = nc.dram_tensor("wdiag", [128, 128], f16, kind="ExternalInput").ap()
    zcol = nc.dram_tensor("zcol", [C, 1], f16, kind="ExternalInput").ap()
    ye = nc.dram_tensor("ye", [BL, C, T2], f16, kind="ExternalOutput").ap()
    yo = nc.dram_tensor("yo", [BL, C, T2], f16, kind="ExternalOutput").ap()

    with tile.TileContext(nc) as tc:
        with (
            tc.tile_pool(name="const", bufs=1) as cpool,
            tc.tile_pool(name="xin", bufs=6) as xpool,
            tc.tile_pool(name="ups", bufs=2, space="PSUM") as ppool,
            tc.tile_pool(name="work", bufs=4) as wpool,
            tc.tile_pool(name="yout", bufs=6) as ypool,
        ):
            # consts ride the (initially idle) ACT ring so the x stream
            # starts immediately on the SP ring
            cwdiag_t = cpool.tile([128, 128], f16, name="cwdiag_t")
            nc.scalar.dma_start(cwdiag_t[:], cwdiag[:])
            wdiag_t = cpool.tile([128, 128], f16, name="wdiag_t")
            nc.scalar.dma_start(wdiag_t[:], wdiag[:])
            zcol_t = cpool.tile([C, 1], f16, name="zcol_t")
            nc.scalar.dma_start(zcol_t[:], zcol[:])
            s0q_t = cpool.tile([C, BL], f32, name="s0q_t")
            nc.scalar.dma_start(s0q_t[:], s0q[:])
            ccol_t = cpool.tile([C, 1], f32, name="ccol_t")
            nc.scalar.dma_start(ccol_t[:], ccol[:])
            wcol_t = cpool.tile([C, 1], f32, name="wcol_t")
            nc.scalar.dma_start(wcol_t[:], wcol[:])
            c2dec_t = cpool.tile([C, J], f16, name="c2dec_t")
            nc.scalar.dma_start(c2dec_t[:], c2dec[:])

            prev_xo = {}
            prev_ye = {}
            pend_e = []
            pend_o = []
            for k in range(NCH):
                for b in range(BL):
                    sl = slice(k * J, (k + 1) * J)
                    xet = xpool.tile([C, J], f16, name=f"xet{b}_{k}", tag="xe")
                    nc.sync.dma_start(xet[:], xe[b][:, sl])
                    xot = xpool.tile([C, J], f16, name=f"xot{b}_{k}", tag="xo")
                    nc.sync.dma_start(xot[:], xo[b][:, sl])

                    # u' = diag(cw) @ x_odd_shifted + diag(w) @ x_even  (PSUM)
                    # each accumulation group is a start/stop pair over an
                    # identical PSUM region (col 0 handled as its own pair)
                    up = ppool.tile([C, J], f32, name="up", tag="up")
                    pcol = zcol_t[:] if k == 0 else prev_xo[b][:, J - 1 : J]
                    nc.tensor.matmul(
                        up[:, 0:1], wdiag_t[:], xet[:, 0:1],
                        start=True, stop=False,
                    )
                    nc.tensor.matmul(
                        up[:, 0:1], cwdiag_t[:], pcol,
                        start=False, stop=True,
                    )
                    for m in range(J // MM):
                        lo, hi = m * MM, (m + 1) * MM
                        lo1 = lo + 1 if m == 0 else lo
                        nc.tensor.matmul(
                            up[:, lo1:hi], wdiag_t[:], xet[:, lo1:hi],
                            start=True, stop=False,
                        )
                        nc.tensor.matmul(
                            up[:, lo1:hi], cwdiag_t[:], xot[:, lo1 - 1 : hi - 1],
                            start=False, stop=True,
                        )

                    # evacuate u' PSUM -> SBUF (f32), then the even scan:
                    # a_{2j} = c^2 * a_{2j-2} + u'[j]
                    ut = wpool.tile([C, J], f16, name="ut", tag="ut")
                    nc.scalar.activation(ut[:], up[:], AF.Copy)
                    yet = ypool.tile([C, J], f16, name=f"yet{b}_{k}", tag="ye")
                    init = (
                        s0q_t[:, b : b + 1]
                        if k == 0
                        else prev_ye[b][:, J - 1 : J]
                    )
                    nc.vector.tensor_tensor_scan(
                        yet[:], c2dec_t[:], ut[:], init,
                        op0=ALU.mult, op1=ALU.add,
                    )

                    # odd reconstruction: y_odd = c*y_even + w*x_odd
                    # single DVE STT keeps everything on nominal-rate paths
                    wxot = wpool.tile([C, J], f16, name="wxot", tag="wxo")
                    nc.scalar.activation(
                        wxot[:], xot[:], AF.Copy, scale=wcol_t[:]
                    )
                    yot = ypool.tile([C, J], f16, name=f"yot{b}_{k}", tag="yo")
                    nc.vector.scalar_tensor_tensor(
                        yot[:], yet[:], ccol_t[:], wxot[:],
                        op0=ALU.mult, op1=ALU.add,
                    )

                    # defer out-DMA issues so no ring ever waits on a
                    # scan/STT still in flight (ye ready before yo)
                    pend_e.append((ye[b][:, sl], yet))
                    pend_o.append((yo[b][:, sl], yot))
                    if len(pend_e) > 2:
                        oye, oyet = pend_e.pop(0)
                        nc.sync.dma_start(oye, oyet[:])
                    if len(pend_o) > 3:
                        oyo, oyot = pend_o.pop(0)
                        nc.sync.dma_start(oyo, oyot[:])

                    prev_xo[b] = xot
                    prev_ye[b] = yet
            for oye, oyet in pend_e:
                nc.sync.dma_start(oye, oyet[:])
            for oyo, oyot in pend_o:
                nc.sync.dma_start(oyo, oyot[:])

    nc.compile()
    _NC_CACHE = nc
    return nc


def _prep(inputs, initial_state, weights):
    x = np.asarray(inputs, dtype=np.float32)
    s0 = np.asarray(initial_state, dtype=np.float32)
    w = np.clip(np.asarray(weights, dtype=np.float32), 0.0, 1.0)
    c = (1.0 - w).astype(np.float32)

    csafe = np.maximum(c, np.float32(1e-30))
    s0q = (s0 / csafe).astype(np.float32)                    # [B, C]
    xT16 = x.transpose(0, 2, 1).astype(np.float16)           # [B, C, T]
    xe = np.ascontiguousarray(xT16[:, :, 0::2])
    xo = np.ascontiguousarray(xT16[:, :, 1::2])
    c2 = (c.astype(np.float64) ** 2).astype(np.float16)
    c2dec = np.ascontiguousarray(np.repeat(c2[:, None], J, axis=1))
    ccol = np.ascontiguousarray(c[:, None])
    wcol = np.ascontiguousarray(w[:, None])
    cwdiag = np.diag((c * w)).astype(np.float16)
    wdiag = np.diag(w).astype(np.float16)
    zcol = np.zeros((C, 1), np.float16)

    maps = []
    for i in range(NCORES):
        sl = slice(i * BL, (i + 1) * BL)
        maps.append(
            {
                "xe": np.ascontiguousarray(xe[sl]),
                "xo": np.ascontiguousarray(xo[sl]),
                "s0q": np.ascontiguousarray(s0q[sl].T),
                "c2dec": c2dec,
                "ccol": ccol,
                "wcol": wcol,
                "cwdiag": cwdiag,
                "wdiag": wdiag,
                "zcol": zcol,
            }
        )
    return maps


def _ensure_ntff_hook():
    """Shim antenv.axon_hooks (absent in this image) so trace=True works."""
    import types

    import antenv

    if not hasattr(antenv, "axon_hooks"):
        mod = types.ModuleType("antenv.axon_hooks")
        holder = [None]
        mod.set_axon_ntff_profile_hook = lambda h: holder.__setitem__(0, h)
        mod.get_axon_ntff_profile_hook = lambda: holder[0]
        sys.modules["antenv.axon_hooks"] = mod
        antenv.axon_hooks = mod
    from antenv.axon_hooks import (
        get_axon_ntff_profile_hook,
        set_axon_ntff_profile_hook,
    )

    if get_axon_ntff_profile_hook() is None:
        from trn_agent_boot.trn_boot import _ntff_profile_via_ctypes

        set_axon_ntff_profile_hook(
            _ntff_profile_via_ctypes("/opt/axon/libaxon_pjrt.so")
        )


def run(inputs, initial_state, weights, trace=False, **kw):
    from concourse import bass_utils

    if trace:
        _ensure_ntff_hook()
    nc = build_bass()
    maps = _prep(inputs, initial_state, weights)
    res = bass_utils.run_bass_kernel_spmd(
        nc, maps, core_ids=list(range(NCORES)), trace=trace, **kw
    )
    yeT = np.concatenate([r["ye"] for r in res.results], axis=0)  # [B, C, T2]
    yoT = np.concatenate([r["yo"] for r in res.results], axis=0)
    yT = np.empty((B, C, T), np.float16)
    yT[:, :, 0::2] = yeT
    yT[:, :, 1::2] = yoT
    out = yT.transpose(0, 2, 1).astype(np.float32)                # [B, T, C]
    return out, res


def kernel(inputs, initial_state, weights):
    out, _ = run(inputs, initial_state, weights)
    return out
